# revision 1
# baseline (speedup 1.0000x reference)
"""Submanifold 3x3x3 sparse conv (gnn_message_passing) + BatchNorm + LeakyReLU
on 8 Trainium2 NeuronCores.

Strategy (hardcoded for N=200000, C=128, K=27, GRID=128^3 @ ~9.5% occupancy):
  * The active-voxel neighbor graph at this occupancy is far below the cubic
    site-percolation threshold, so it splits into ~31k tiny connected
    components (max ~2.4k voxels). We partition whole components across the
    8 cores (LPT bin packing) -> every neighbor reference stays inside its
    core's shard. No halo exchange, and shard-local indices fit in int16,
    which is what the SWDGE dma_gather ucode requires.
  * Per core: the shard's features live as a bf16 token table in SBUF.
    For each 512-row supertile, ONE merged dma_gather (transpose=True)
    gathers all 26 non-self neighbor rows k-major -> a [128, 26*512] bf16
    tile that is directly the transposed matmul rhs. The self offset (k=13)
    is a contiguous slice streamed from a host-pretransposed table.
  * 27 accumulating bf16 matmuls (lhsT = W[k], C_in on partitions) produce
    the conv output transposed [C_out, 512] in fp32 PSUM.
  * BN statistics: per-supertile DVE reduce (sum) + ACT Square with
    accum_out (sum of squares), finalized and all-reduced across the 8
    cores with one tiny AllReduce collective. b is ignored: BatchNorm is
    shift-invariant so the conv bias cancels exactly.
  * BN apply + LeakyReLU is a single ACT Lrelu instruction per tile
    (out = lrelu(x*scale + shift), per-partition scale/shift), then PE
    transposes back to row-major and contiguous DMA writeback.
  * Host reassembles shards and inverts the component permutation.

Falls back to a pure-numpy reference computation if the input graph is not
separable into <=25088-row shards (never the case for the intended input
distribution).
"""

import numpy as np
import ml_dtypes

C = 128
K = 27
EPS = 1e-4
LEAK = 0.333
N_CORES = 8
SELF_K = 13

F32 = None  # set lazily after concourse import
BF16 = None
I16 = None


class Cfg:
    def __init__(self, n_total, st, n_st, table_rows, n_cores):
        assert st % 128 == 0 and table_rows % 128 == 0
        self.n_total = n_total          # global number of real rows (stats divisor)
        self.st = st                    # supertile rows
        self.n_st = n_st                # supertiles per core
        self.shard = st * n_st          # padded rows per core
        self.table_rows = table_rows    # shard table rows incl. zero pad
        self.ranks = table_rows // 128
        self.zero_row = table_rows - 1
        self.n_cores = n_cores
        self.kg = K - 1                 # gathered (non-self) offsets
        self.merged = self.kg * st      # idxs per merged gather
        self.idx_cols = self.merged // 16
        assert self.merged % 128 == 0
        # Chunked single-packet gathers: 512 idxs = 32 descs/engine (8KB
        # packet) verified working; 1024 idxs (16KB packet) crashes the
        # exec unit; single_packet=False (per-desc packets) is ~2x slower
        # end-to-end. HW-measured 2026-08-04.
        self.gather_chunk = 512 if self.merged % 512 == 0 else (
            256 if self.merged % 256 == 0 else 0)


FULL_CFG = Cfg(n_total=200_000, st=512, n_st=49, table_rows=25_216, n_cores=N_CORES)


def emit_kernel(tc, out_ap, ins, cfg):
    """Emit the per-core program. ins: dict with APs for
    table [table_rows, C] bf16, table_t [C, shard] bf16,
    idx [128, n_st*idx_cols] int16, w [K, C, C] bf16,
    gamma [C] f32, beta [C] f32. out_ap: [shard, C] f32."""
    import concourse.mybir as mybir
    from concourse.bass import ts
    from concourse.masks import make_identity

    nc = tc.nc
    F32 = mybir.dt.float32
    BF16 = mybir.dt.bfloat16
    I16 = mybir.dt.int16
    ST, N_ST = cfg.st, cfg.n_st
    NB = ST // 128  # row blocks per supertile

    table, table_t, idx, w = ins["table"], ins["table_t"], ins["idx"], ins["w"]
    gamma, beta = ins["gamma"], ins["beta"]

    # order of gathered offsets in the merged index list
    kg_list = [k for k in range(K) if k != SELF_K]

    with (
        tc.tile_pool(name="const", bufs=1) as constp,
        tc.tile_pool(name="gath", bufs=2) as gathp,
        tc.tile_pool(name="selfp", bufs=2) as selfp,
        tc.tile_pool(name="idxp", bufs=2) as idxp,
        tc.tile_pool(name="work", bufs=2) as workp,
        tc.tile_pool(name="psum", bufs=2, space="PSUM") as psump,
        tc.tile_pool(name="psumt", bufs=2, space="PSUM") as psumtp,
        tc.tile_pool(name="dram", bufs=1, space="DRAM") as dramp,
    ):
        table_sb = constp.tile([128, cfg.ranks * C], BF16)
        nc.sync.dma_start(table_sb[:].rearrange("p (r c) -> p r c", r=cfg.ranks),
                          table.rearrange("(r p) c -> p r c", p=128))
        w_sb = constp.tile([128, K * C], BF16)
        nc.sync.dma_start(w_sb[:].rearrange("ci (k co) -> ci k co", k=K),
                          w.rearrange("k ci co -> ci k co"))
        gamma_sb = constp.tile([128, 1], F32)
        nc.sync.dma_start(gamma_sb[:], gamma[:, None])
        beta_sb = constp.tile([128, 1], F32)
        nc.sync.dma_start(beta_sb[:], beta[:, None])
        identity = constp.tile([128, 128], F32)
        make_identity(nc, identity[:])

        out_t = constp.tile([128, cfg.shard], BF16)   # staged pre-BN, transposed
        sum_part = constp.tile([128, N_ST], F32)
        sq_part = constp.tile([128, N_ST], F32)
        gt_probe = None
        if getattr(cfg, "skip_gather", False):  # perf probe only
            gt_probe = constp.tile([128, 1, cfg.merged], BF16)
            nc.vector.memset(gt_probe[:, :, :], 0)

        # ---- phase 1: conv + stats ----
        for s in range(N_ST):
            it = idxp.tile([128, cfg.idx_cols], I16)
            nc.sync.dma_start(it[:], idx[:, s * cfg.idx_cols:(s + 1) * cfg.idx_cols])
            chunk = getattr(cfg, "gather_chunk", 0)
            if gt_probe is not None:  # perf probe only
                gt = gt_probe
            else:
                gt = gathp.tile([128, 1, cfg.merged], BF16)
            if gt_probe is not None:
                pass
            elif chunk:
                # single_packet=True needs <=1024 idxs (64 descs/engine =
                # one packet); chunked gathers keep packets maximal.
                assert cfg.merged % chunk == 0 and chunk <= 1024
                cw = chunk // 16
                for gc in range(cfg.merged // chunk):
                    nc.gpsimd.dma_gather(
                        gt[:, :, gc * chunk:(gc + 1) * chunk], table_sb[:],
                        it[:, gc * cw:(gc + 1) * cw], chunk, chunk, C,
                        transpose=True,
                        single_packet=True,
                        sbuf_tokens_per_rank=128,
                        sbuf_free_dim_per_rank=C * 2,
                    )
            else:
                nc.gpsimd.dma_gather(
                    gt[:, :, :], table_sb[:], it[:], cfg.merged, cfg.merged, C,
                    transpose=True,
                    single_packet=False,
                    sbuf_tokens_per_rank=128,
                    sbuf_free_dim_per_rank=C * 2,
                )
            st_self = selfp.tile([128, ST], BF16)
            nc.sync.dma_start(st_self[:], table_t[:, s * ST:(s + 1) * ST])

            ps = psump.tile([128, ST], F32)
            for kk in range(K):
                if kk == SELF_K:
                    rhs = st_self[:]
                else:
                    kidx = kg_list.index(kk)
                    rhs = gt[:, 0, ts(kidx, ST)]
                nc.tensor.matmul(ps[:], w_sb[:, ts(kk, C)], rhs,
                                 start=(kk == 0), stop=(kk == K - 1))

            nc.vector.reduce_sum(out=sum_part[:, s:s + 1], in_=ps[:],
                                 axis=mybir.AxisListType.X)
            trash = workp.tile([128, ST], F32)
            nc.scalar.activation(trash[:], ps[:],
                                 mybir.ActivationFunctionType.Square,
                                 accum_out=sq_part[:, s:s + 1])
            nc.vector.tensor_copy(out_t[:, s * ST:(s + 1) * ST], ps[:])

        # ---- stats finalize + all-reduce ----
        stats_sb = constp.tile([128, 2], F32)
        nc.vector.reduce_sum(out=stats_sb[:, 0:1], in_=sum_part[:],
                             axis=mybir.AxisListType.X)
        nc.vector.reduce_sum(out=stats_sb[:, 1:2], in_=sq_part[:],
                             axis=mybir.AxisListType.X)

        if cfg.n_cores > 1 and not getattr(cfg, "skip_collective", False):
            stats_in = dramp.tile([128, 2], F32)
            stats_out = dramp.tile([128, 2], F32)
            nc.sync.dma_start(stats_in[:], stats_sb[:])
            nc.gpsimd.collective_compute(
                "AllReduce", mybir.AluOpType.add,
                replica_groups=[list(range(cfg.n_cores))],
                ins=[stats_in.opt()], outs=[stats_out.opt()],
            )
            stats2_sb = constp.tile([128, 2], F32)
            nc.sync.dma_start(stats2_sb[:], stats_out[:])
        else:
            stats2_sb = stats_sb

        mean_t = constp.tile([128, 1], F32)
        ex2_t = constp.tile([128, 1], F32)
        var_t = constp.tile([128, 1], F32)
        std_t = constp.tile([128, 1], F32)
        rstd_t = constp.tile([128, 1], F32)
        s_vec = constp.tile([128, 1], F32)
        t_vec = constp.tile([128, 1], F32)
        tmp = constp.tile([128, 1], F32)
        inv_n = 1.0 / cfg.n_total
        nc.vector.tensor_scalar_mul(mean_t[:], stats2_sb[:, 0:1], inv_n)
        nc.vector.tensor_scalar_mul(ex2_t[:], stats2_sb[:, 1:2], inv_n)
        nc.vector.tensor_tensor(out=tmp[:], in0=mean_t[:], in1=mean_t[:],
                                op=mybir.AluOpType.mult)
        nc.vector.tensor_tensor(out=var_t[:], in0=ex2_t[:], in1=tmp[:],
                                op=mybir.AluOpType.subtract)
        nc.vector.tensor_scalar_add(var_t[:], var_t[:], EPS)
        nc.scalar.activation(std_t[:], var_t[:],
                             mybir.ActivationFunctionType.Sqrt)
        nc.vector.reciprocal(rstd_t[:], std_t[:])
        nc.vector.tensor_tensor(out=s_vec[:], in0=rstd_t[:], in1=gamma_sb[:],
                                op=mybir.AluOpType.mult)
        nc.vector.tensor_tensor(out=tmp[:], in0=mean_t[:], in1=s_vec[:],
                                op=mybir.AluOpType.mult)
        nc.vector.tensor_tensor(out=t_vec[:], in0=beta_sb[:], in1=tmp[:],
                                op=mybir.AluOpType.subtract)

        # ---- phase 2: BN + LeakyReLU + transpose back + writeback ----
        for s in range(N_ST):
            bn = workp.tile([128, ST], F32)
            nc.scalar.activation(bn[:], out_t[:, s * ST:(s + 1) * ST],
                                 mybir.ActivationFunctionType.Identity,
                                 bias=t_vec[:, 0:1], scale=s_vec[:, 0:1])
            bn2 = workp.tile([128, ST], F32)
            nc.vector.tensor_scalar_mul(bn2[:], bn[:], LEAK)
            nc.vector.tensor_tensor(out=bn[:], in0=bn[:], in1=bn2[:],
                                    op=mybir.AluOpType.max)
            pt = psumtp.tile([128, ST], F32)
            for b in range(NB):
                nc.tensor.transpose(pt[:, ts(b, 128)], bn[:, ts(b, 128)],
                                    identity[:])
            stage = workp.tile([128, ST], F32)
            nc.vector.tensor_copy(stage[:], pt[:])
            nc.sync.dma_start(
                out_ap[s * ST:(s + 1) * ST, :].rearrange("(b p) c -> p b c", p=128),
                stage[:].rearrange("p (b c) -> p b c", b=NB),
            )


# ----------------------------------------------------------------------------
# host-side preparation
# ----------------------------------------------------------------------------

def _partition_components(nb, n, n_cores, shard_cap):
    """Whole-component LPT partition. Returns (members_per_core, ok)."""
    import scipy.sparse as sp
    import scipy.sparse.csgraph as csg
    import heapq

    valid = nb >= 0
    ii, kk = np.nonzero(valid)
    jj = nb[ii, kk]
    m = kk != SELF_K
    g = sp.coo_matrix((np.ones(m.sum(), np.int8), (ii[m], jj[m])), shape=(n, n))
    _, labels = csg.connected_components(g, directed=False)
    sizes = np.bincount(labels)
    if sizes.max() > shard_cap:
        return None, False
    order = np.argsort(sizes)[::-1]
    heap = [(0, c) for c in range(n_cores)]
    heapq.heapify(heap)
    assign = np.empty(len(sizes), np.int32)
    for comp in order:
        load, c = heapq.heappop(heap)
        assign[comp] = c
        heapq.heappush(heap, (load + int(sizes[comp]), c))
    if max(l for l, _ in heap) > shard_cap:
        return None, False
    shard_of = assign[labels]
    members = [np.nonzero(shard_of == c)[0] for c in range(n_cores)]
    return members, True


def _prepare_core_inputs(features, nb, members, cfg):
    n = features.shape[0]
    loc = np.full(n, cfg.zero_row, np.int32)
    for mem in members:
        loc[mem] = np.arange(len(mem), dtype=np.int32)

    kg_list = [k for k in range(K) if k != SELF_K]
    in_maps = []
    for mem in members:
        real = len(mem)
        assert real <= cfg.shard
        table = np.zeros((cfg.table_rows, C), ml_dtypes.bfloat16)
        table[:real] = features[mem].astype(ml_dtypes.bfloat16)
        table_t = np.ascontiguousarray(
            table[:cfg.shard].T)  # [C, shard] bf16

        idx16 = np.full((cfg.shard, cfg.kg), cfg.zero_row, np.int32)
        nb_c = nb[mem][:, kg_list]                   # [real, kg]
        v = nb_c >= 0
        li = loc[np.where(v, nb_c, 0)]
        assert (li[v] < real).all(), "neighbor escaped shard"
        idx16[:real] = np.where(v, li, cfg.zero_row)
        idx16 = idx16.astype(np.int16)

        idx_dram = np.empty((128, cfg.n_st * cfg.idx_cols), np.int16)
        for s in range(cfg.n_st):
            m = idx16[s * cfg.st:(s + 1) * cfg.st, :].T.reshape(-1)  # k-major
            wrapped = m.reshape(cfg.idx_cols, 16).T                  # [16, cols]
            idx_dram[:, s * cfg.idx_cols:(s + 1) * cfg.idx_cols] = \
                np.tile(wrapped, (8, 1))
        in_maps.append({"table": table, "table_t": table_t, "idx": idx_dram})
    return in_maps


def _reference_fallback(features, w, b, gamma, beta, nb):
    feats = np.asarray(features, np.float32)
    wf = np.asarray(w, np.float32)
    out = np.broadcast_to(np.asarray(b, np.float32), feats.shape).copy()
    valid = nb >= 0
    idx = np.where(valid, nb, 0)
    for k in range(K):
        xk = feats[idx[:, k]] * valid[:, k:k + 1]
        out += xk @ wf[k]
    mean = out.mean(0)
    var = out.var(0)
    out = (out - mean) / np.sqrt(var + EPS) * np.asarray(gamma, np.float32) \
        + np.asarray(beta, np.float32)
    return np.where(out > 0, out, LEAK * out).astype(np.float32)


def _build_bass(cfg, reps=1):
    import concourse.bacc as bacc
    import concourse.mybir as mybir
    import concourse.tile as tile

    nc = bacc.Bacc("TRN2", target_bir_lowering=False, debug=False,
                   num_devices=cfg.n_cores)
    F32 = mybir.dt.float32
    BF16 = mybir.dt.bfloat16
    I16 = mybir.dt.int16
    ins = {
        "table": nc.dram_tensor("table", [cfg.table_rows, C], BF16,
                                kind="ExternalInput")[:, :],
        "table_t": nc.dram_tensor("table_t", [C, cfg.shard], BF16,
                                  kind="ExternalInput")[:, :],
        "idx": nc.dram_tensor("idx", [128, cfg.n_st * cfg.idx_cols], I16,
                              kind="ExternalInput")[:, :],
        "w": nc.dram_tensor("w", [K, C, C], BF16, kind="ExternalInput")[:, :, :],
        "gamma": nc.dram_tensor("gamma", [C], F32, kind="ExternalInput")[:],
        "beta": nc.dram_tensor("beta", [C], F32, kind="ExternalInput")[:],
    }
    out = nc.dram_tensor("out", [cfg.shard, C], F32, kind="ExternalOutput")
    with tile.TileContext(nc) as tc:
        for _ in range(reps):
            emit_kernel(tc, out[:, :], ins, cfg)
    nc.compile()
    return nc


def kernel(features, W, b, gamma, beta, neighbor_idx):
    from concourse.bass_utils import run_bass_kernel_spmd

    features = np.asarray(features, np.float32)
    Wf = np.asarray(W, np.float32)
    gamma_f = np.asarray(gamma, np.float32)
    beta_f = np.asarray(beta, np.float32)
    nb = np.asarray(neighbor_idx, np.int32)
    cfg = FULL_CFG
    assert features.shape == (cfg.n_total, C)

    members, ok = _partition_components(nb, cfg.n_total, cfg.n_cores, cfg.shard)
    if not ok:
        return _reference_fallback(features, Wf, b, gamma_f, beta_f, nb)

    core_maps = _prepare_core_inputs(features, nb, members, cfg)
    w_bf = Wf.astype(ml_dtypes.bfloat16)
    for m in core_maps:
        m["w"] = w_bf
        m["gamma"] = gamma_f
        m["beta"] = beta_f

    nc = _build_bass(cfg)
    res = run_bass_kernel_spmd(nc, core_maps, core_ids=list(range(cfg.n_cores)))

    out_full = np.empty((cfg.n_total, C), np.float32)
    for c, mem in enumerate(members):
        out_full[mem] = res.results[c]["out"][:len(mem)]
    return out_full

